# revision 23
# baseline (speedup 1.0000x reference)
"""Causal self-attention (B=4, T=2048, C=1024, H=16, D=64) on 8 trn2 NeuronCores.

Sharding: core c = (batch b = c//2, head-group g = c%2). Megatron-style within a
batch: each core computes 8 heads' q/k/v (column-parallel) and a row-parallel
partial out-projection. Host sums the two partials per batch and adds the
rank-1 bias term (bo + bv @ wo) -- valid because softmax rows sum to 1, so v's
bias never needs to enter the kernel.

Per-core kernel (all matmuls bf16, fp32 PSUM accumulation):
  phase 1 (per 512-wide T chunk): qT,kT = (x@w)^T via lhsT=w, rhs=x^T (host
           pre-transposes x); v natural via lhsT=x^T-chunk, rhs=wv; a ones
           column is appended to each head's v block.
  phase 2: flash-style streaming attention in S^T orientation:
           S^T[k,q] = kT.T @ qT (head pairs packed in PE row groups 0/64,
           two concurrent 64x128 row tiles); P^T = exp(S^T) (ScalarE,
           1/sqrt(D) folded into q), causal mask-multiply on diagonal tiles
           only; O^T accumulated via lhsT=v_tile, rhs=P^T; the ones column
           of v makes PSUM row 64 the softmax denominator Z for free.
  phase 3: y = O @ wo via lhsT=O^T (already the natural layout), rhs=wo.

Scheduling: the PE sequencer is FIFO, so the span is governed by program
order, not just dependencies. The attention EXP stream (ScalarE, ~930ns/ik)
is the pace-setter; per-ik the PE has ~300ns of slack. All projection /
out-projection work is therefore chopped into small "filler units" (one
PSUM accumulation group each) that are drained one-at-a-time between
attention iks, keeping both engines dense. Chunk-0 q/k/v is fused into the
jq=0 pair loop (ramp), and the final chunk's out-projection accumulates
t=0..2 into SBUF mid-attention so only the t=3 term remains in the tail.
S PSUM uses 3 rotating slots so S^T(ik) never waits on exp(ik-2).
"""
import numpy as np
import ml_dtypes

import concourse.tile as tile
from concourse import bacc, mybir
from concourse.bass_utils import run_bass_kernel_spmd

BF16 = ml_dtypes.bfloat16
F32 = mybir.dt.float32
BT16 = mybir.dt.bfloat16
AF = mybir.ActivationFunctionType
ALU = mybir.AluOpType

B, T, C, H, D = 4, 2048, 1024, 16, 64
G = 2              # head groups (cores per batch)
HL = H // G        # heads per core = 8
HD = HL * D        # local head dims = 512
NP = 4             # head pairs per core
NJQ = T // 512     # q chunks of 512 = 4
NIK = T // 128     # k tiles of 128 = 16
KC = C // 128      # contraction chunks = 8

_CACHED = {}


def _build():
    nc = bacc.Bacc("TRN2", debug=False)
    xT = nc.dram_tensor("xT", [NJQ, 128, KC, 512], BT16, kind="ExternalInput").ap()
    wq = nc.dram_tensor("wq", [128, KC, HD], BT16, kind="ExternalInput").ap()
    wk = nc.dram_tensor("wk", [128, KC, HD], BT16, kind="ExternalInput").ap()
    wv = nc.dram_tensor("wv", [128, KC, HD], BT16, kind="ExternalInput").ap()
    wo = nc.dram_tensor("wo", [128, NP, C], BT16, kind="ExternalInput").ap()
    wo3h = nc.dram_tensor("wo3h", [64, C], BT16, kind="ExternalInput").ap()
    bq = nc.dram_tensor("bq", [128, NP], F32, kind="ExternalInput").ap()
    bk = nc.dram_tensor("bk", [128, NP], F32, kind="ExternalInput").ap()
    masks = nc.dram_tensor("masks", [128, 4, 512], BT16, kind="ExternalInput").ap()
    rcp_dram = nc.dram_tensor("rcp_dram", [NJQ, 8, 512], BT16).ap()
    y = nc.dram_tensor("y", [T, C], BT16, kind="ExternalOutput").ap()

    with tile.TileContext(nc) as tc:
        with (
            tc.tile_pool(name="consts", bufs=1) as consts,
            tc.tile_pool(name="xt", bufs=3) as xtp,
            tc.tile_pool(name="qk", bufs=1) as qkp,
            tc.tile_pool(name="vp", bufs=1) as vp,
            tc.tile_pool(name="otp", bufs=1) as otp,
            tc.tile_pool(name="pt", bufs=6) as ptp,
            tc.tile_pool(name="ptmp", bufs=3) as ptmpp,
            tc.tile_pool(name="zn", bufs=3) as znp,
            tc.tile_pool(name="yst", bufs=4) as ystp,
            tc.tile_pool(name="ps", bufs=3, space="PSUM") as ps,
        ):
            # ---- constants (biases are tiny and gate evictions: load them first) ----
            bq_dma = consts.tile([128, NP], F32, tag="bq_dma")
            bq_sb = consts.tile([128, NP], F32, tag="bq")
            nc.sync.dma_start(bq_dma, bq)
            nc.vector.tensor_copy(bq_sb, bq_dma)
            bk_dma = consts.tile([128, NP], F32, tag="bk_dma")
            bk_sb = consts.tile([128, NP], F32, tag="bk")
            nc.sync.dma_start(bk_dma, bk)
            nc.vector.tensor_copy(bk_sb, bk_dma)
            wq_sb = consts.tile([128, KC, HD], BT16, tag="wq")
            xt0 = xtp.tile([128, KC, 512], BT16, tag="xt", name="xt_pre0")
            wk_sb = consts.tile([128, KC, HD], BT16, tag="wk")
            for h in range(4):
                ks = slice(2 * h, 2 * h + 2)
                nc.scalar.dma_start(wq_sb[:, ks, :], wq[:, ks, :])
                nc.sync.dma_start(xt0[:, ks, :], xT[0][:, ks, :])
                nc.gpsimd.dma_start(wk_sb[:, ks, :], wk[:, ks, :])
            wv_sb = consts.tile([128, KC, HD], BT16, tag="wv")
            for h in range(2):
                ks = slice(4 * h, 4 * h + 4)
                nc.gpsimd.dma_start(wv_sb[:, ks, :], wv[:, ks, :])
            masks_sb = consts.tile([128, 4, 512], BT16, tag="masks")
            nc.gpsimd.dma_start(masks_sb, masks)
            wo_sb = consts.tile([128, NP, C], BT16, tag="wo")
            for h in range(2):
                nc.scalar.dma_start(wo_sb[:, 2 * h:2 * h + 2, :], wo[:, 2 * h:2 * h + 2, :])
            wo3h_sb = consts.tile([64, C], BT16, tag="wo3h")
            nc.scalar.dma_start(wo3h_sb, wo3h)
            # ---- persistent activations ----
            qT = [qkp.tile([128, T], BT16, tag=f"qT{t}", name=f"qT{t}") for t in range(NP)]
            kT = [qkp.tile([128, T], BT16, tag=f"kT{t}", name=f"kT{t}") for t in range(NP)]
            v_sb = [vp.tile([128, HL * 65], BT16, tag=f"v{i}", name=f"v{i}") for i in range(NIK)]
            oT = [otp.tile([128, T], BT16, tag=f"oT{t}", name=f"oT{t}") for t in range(NP)]
            yacc = {}  # (m, n) -> SBUF partial sum over t=0..2 for the last chunk

            # ---- phase 1 building blocks ----
            def q_unit(jt, t, xt):
                p = ps.tile([128, 512], F32, tag="st", name=f"pq{jt}_{t}")
                for k in range(KC):
                    nc.tensor.matmul(
                        p, wq_sb[:, k, t * 128:(t + 1) * 128], xt[:, k, :],
                        start=(k == 0), stop=(k == KC - 1),
                    )
                nc.vector.tensor_scalar(
                    qT[t][:, jt * 512:(jt + 1) * 512], p,
                    0.125, bq_sb[:, t:t + 1], ALU.mult, ALU.add,
                )

            def k_unit(jt, t, xt):
                p = ps.tile([128, 512], F32, tag="st", name=f"pk{jt}_{t}")
                for k in range(KC):
                    nc.tensor.matmul(
                        p, wk_sb[:, k, t * 128:(t + 1) * 128], xt[:, k, :],
                        start=(k == 0), stop=(k == KC - 1),
                    )
                nc.vector.tensor_scalar_add(
                    kT[t][:, jt * 512:(jt + 1) * 512], p, bk_sb[:, t:t + 1]
                )

            def v_unit(jt, s, xt):
                ik = jt * 4 + s
                p = ps.tile([128, 512], F32, tag="st", name=f"pv{ik}")
                for k in range(KC):
                    nc.tensor.matmul(
                        p, xt[:, k, s * 128:(s + 1) * 128], wv_sb[:, k, :],
                        start=(k == 0), stop=(k == KC - 1),
                    )
                vg = v_sb[ik].rearrange("p (h c) -> p h c", c=65)
                nc.vector.tensor_copy(
                    vg[:, :, 0:64], p.rearrange("p (h c) -> p h c", c=64)
                )
                nc.vector.memset(vg[:, :, 64:65], 1.0)

            def xt_load(jt):
                xt = xtp.tile([128, KC, 512], BT16, tag="xt", name=f"xt{jt}")
                nc.sync.dma_start(xt[:, 0:4, :], xT[jt][:, 0:4, :])
                nc.sync.dma_start(xt[:, 4:8, :], xT[jt][:, 4:8, :])
                return xt

            # ---- phase 3 building blocks ----
            def p3_unit(jq3, m, n):
                p = ps.tile([128, 512], F32, tag="st", name=f"py{m}_{n}")
                for t in range(NP):
                    nc.tensor.matmul(
                        p, oT[t][:, m * 128:(m + 1) * 128],
                        wo_sb[:, t, n * 512:(n + 1) * 512],
                        start=(t == 0), stop=(t == NP - 1),
                    )
                ys = ystp.tile([128, 512], BT16, tag="y", name=f"ys{m}_{n}")
                nc.vector.tensor_copy(ys, p)
                nc.gpsimd.dma_start(
                    y[m * 128:(m + 1) * 128, n * 512:(n + 1) * 512], ys
                )

            def p3_part_unit(m, n):
                # last chunk: accumulate t=0..2 into SBUF while pair 3's
                # attention still runs; t=3's term lands in the tail
                p = ps.tile([128, 512], F32, tag="st", name=f"pyp{m}_{n}")
                for t in range(NP - 1):
                    nc.tensor.matmul(
                        p, oT[t][:, m * 128:(m + 1) * 128],
                        wo_sb[:, t, n * 512:(n + 1) * 512],
                        start=(t == 0), stop=(t == NP - 2),
                    )
                ya = ystp.tile([128, 512], F32, tag=f"ya{m}_{n}", bufs=1, name=f"ya{m}_{n}")
                nc.vector.tensor_copy(ya, p)
                yacc[(m, n)] = ya

            def p3_final_unit(m, n, tmp, dmae=None):
                # t=3 term: head 6 from oT (written lane-aligned), head 7
                # straight from the normalize tmp tile via a low-partition
                # copy of wo's h7 rows -- no partition-shift DMA in the tail
                p = ps.tile([128, 512], F32, tag="st", name=f"pyf{m}_{n}")
                t = NP - 1
                nc.tensor.matmul(
                    p, oT[t][0:64, m * 128:(m + 1) * 128],
                    wo_sb[0:64, t, n * 512:(n + 1) * 512],
                    start=True, stop=False,
                )
                nc.tensor.matmul(
                    p, tmp, wo3h_sb[:, n * 512:(n + 1) * 512],
                    start=False, stop=True,
                )
                ys = ystp.tile([128, 512], BT16, tag="y", name=f"ysf{m}_{n}")
                nc.vector.tensor_add(ys, yacc[(m, n)], p)
                (dmae or nc.gpsimd).dma_start(
                    y[m * 128:(m + 1) * 128, n * 512:(n + 1) * 512], ys
                )

            # ---- filler queue: (chunk_gate, fn); gate=None for phase3 ----
            filler = []
            held = []   # units reserved for the tail's DMA-latency window

            def drain(n):
                for _ in range(n):
                    if filler:
                        filler.pop(0)[1]()

            def flush_chunk(jt):
                i = 0
                while i < len(filler):
                    gate, fn = filler[i]
                    if gate is not None and gate <= jt:
                        filler.pop(i)
                        fn()
                    else:
                        i += 1

            def phase1_enqueue(jt):
                xt = xt_load(jt)
                for t in range(NP):
                    filler.append((jt, lambda t=t, xt=xt: q_unit(jt, t, xt)))
                    filler.append((jt, lambda t=t, xt=xt: k_unit(jt, t, xt)))
                for s in range(4):
                    filler.append((jt, lambda s=s, xt=xt: v_unit(jt, s, xt)))

            # ---- attention ----
            def av(t, ik, nik, pts, o_ps):
                pt, c0 = pts[ik]
                ptg = pt.rearrange("p (h q) -> p h q", q=512)
                for hh in range(2):
                    h = 2 * t + hh
                    nc.tensor.matmul(
                        o_ps[hh][:, c0:512], v_sb[ik][:, h * 65:h * 65 + 65],
                        ptg[:, hh, c0:512],
                        start=(ik == 0), stop=(ik == nik - 1),
                    )

            def attention(t, jq):
                nik = 4 * jq + 4
                o_ps = [
                    ps.tile([65, 512], F32, tag="ot", bufs=2, name=f"ops{t}_{jq}_{_h}")
                    for _h in range(2)
                ]
                pts = {}
                for ik in range(nik):
                    d = ik - 4 * jq
                    c0 = 128 * d if d > 0 else 0   # first potentially-valid column
                    st = ps.tile([128, 1024], F32, tag="st", name=f"st{t}_{jq}_{ik}")
                    stg = st.rearrange("p (h q) -> p h q", q=512)
                    for hh in range(2):
                        r = slice(hh * 64, hh * 64 + 64)
                        nc.tensor.matmul(
                            stg[:, hh, c0:512],
                            kT[t][r, ik * 128:(ik + 1) * 128],
                            qT[t][r, jq * 512 + c0:(jq + 1) * 512],
                            start=True, stop=True,
                        )
                    pt = ptp.tile([128, 1024], BT16, tag="pt", name=f"pt{t}_{jq}_{ik}")
                    ptg = pt.rearrange("p (h q) -> p h q", q=512)
                    if d >= 0:
                        ptm = ptmpp.tile([128, 1024], BT16, tag="ptmp", name=f"ptm{t}_{jq}_{ik}")
                        ptmg = ptm.rearrange("p (h q) -> p h q", q=512)
                        nc.scalar.activation(ptmg[:, :, c0:512], stg[:, :, c0:512], AF.Exp)
                        for hh in range(2):
                            nc.vector.tensor_mul(
                                ptg[:, hh, c0:512],
                                ptmg[:, hh, c0:512],
                                masks_sb[:, d, c0:512],
                            )
                    else:
                        nc.scalar.activation(pt, st, AF.Exp)
                    pts[ik] = (pt, c0)
                    if ik > 0:
                        av(t, ik - 1, nik, pts, o_ps)
                    if ik % 2 == 1 and (jq < 2 or ik >= 4):
                        drain(1 if jq >= 2 else 2)
                av(t, nik - 1, nik, pts, o_ps)
                # evict Z row + unnormalized O^T, freeing the PSUM accumulators
                out_h = []
                for hh in range(2):
                    ouz = znp.tile([65, 512], F32, tag="ouz", bufs=6, name=f"oz{t}_{jq}_{hh}")
                    nc.vector.tensor_copy(ouz[64:65, :], o_ps[hh][64:65, :])
                    nc.vector.tensor_copy(ouz[0:64, :], o_ps[hh][0:64, :])
                    out_h.append(ouz)
                return out_h

            import concourse.bass as bass_mod

            def normalize(t, jq, evicted):
                # Pack both heads' Z rows [1,512] as [8,64] each -> one [16,64]
                # reciprocal (64 elems/lane), then broadcast 1/Z via a DRAM
                # round-trip (partition-step-0 DMA reads are legal from DRAM).
                qs2 = slice(jq * 512, (jq + 1) * 512)
                zb = znp.tile([16, 64], F32, tag="zb", bufs=2, name=f"zb{t}_{jq}")
                for hh in range(2):
                    ouz = evicted[hh]
                    nc.sync.dma_start(
                        zb[8 * hh:8 * hh + 8, :],
                        ouz[64:65, :].rearrange("o (p q) -> o p q", p=8),
                    )
                rcp = znp.tile([16, 64], F32, tag="rcpb", bufs=2, name=f"rcp{t}_{jq}")
                nc.vector.reciprocal(rcp, zb)
                rcp16 = znp.tile([16, 64], BT16, tag="rcp16b", bufs=2, name=f"rcp16{t}_{jq}")
                nc.vector.tensor_copy(rcp16, rcp)
                for hh in range(2):
                    nc.sync.dma_start(
                        rcp_dram[jq, 2 * t + hh, :].rearrange("(p q) -> p q", p=8),
                        rcp16[8 * hh:8 * hh + 8, :],
                    )
                for hh in range(2):
                    ouz = evicted[hh]
                    bc_sb = znp.tile([64, 512], BT16, tag="bc_sb", bufs=3, name=f"bs{t}_{jq}_{hh}")
                    src = rcp_dram[jq, 2 * t + hh, :]
                    bcast = bass_mod.AP(
                        tensor=src.tensor, offset=src.offset,
                        ap=[[0, 64]] + [list(a) for a in src.ap],
                    )
                    nc.sync.dma_start(bc_sb, bcast)
                    if hh == 0:
                        nc.vector.tensor_mul(oT[t][0:64, qs2], ouz[0:64, :], bc_sb)
                    else:
                        tmp = znp.tile([64, 512], BT16, tag="tmp_o", bufs=2, name=f"tm{t}_{jq}")
                        nc.vector.tensor_mul(tmp, ouz[0:64, :], bc_sb)
                        nc.gpsimd.dma_start(oT[t][64:128, qs2], tmp)

            def normalize_final(t, jq, evicted):
                # Last pair of the last chunk: same 1/Z machinery as
                # normalize() (Ln/Exp live in different ACT table sets, so
                # the DMA round-trip beats two table switches), but with the
                # oT muls done per-128-col slice so each slice's t=3
                # out-projection matmul + y writeback pipelines behind the
                # partition-shift DMA of the previous slice; DMAs alternate
                # across the three queues.
                zb = znp.tile([16, 64], F32, tag="zb", bufs=2, name=f"zbF{t}_{jq}")
                for hh in range(2):
                    nc.scalar.dma_start(
                        zb[8 * hh:8 * hh + 8, :],
                        evicted[hh][64:65, :].rearrange("o (p q) -> o p q", p=8),
                    )
                rcp = znp.tile([16, 64], F32, tag="rcpb", bufs=2, name=f"rcpF{t}_{jq}")
                nc.vector.reciprocal(rcp, zb)
                rcp16 = znp.tile([16, 64], BT16, tag="rcp16b", bufs=2, name=f"rcp16F{t}_{jq}")
                nc.vector.tensor_copy(rcp16, rcp)
                for hh in range(2):
                    nc.scalar.dma_start(
                        rcp_dram[jq, 2 * t + hh, :].rearrange("(p q) -> p q", p=8),
                        rcp16[8 * hh:8 * hh + 8, :],
                    )
                drain(len(filler))
                for fn in held:   # PE work covering the DMA latency window
                    fn()
                held.clear()
                bcs = []
                for hh in range(2):
                    bc_sb = znp.tile([64, 512], BT16, tag="bc_sb", bufs=3, name=f"bsF{t}_{jq}_{hh}")
                    src = rcp_dram[jq, 2 * t + hh, :]
                    bcast = bass_mod.AP(
                        tensor=src.tensor, offset=src.offset,
                        ap=[[0, 64]] + [list(a) for a in src.ap],
                    )
                    nc.scalar.dma_start(bc_sb, bcast)
                    bcs.append(bc_sb)
                rings = [nc.sync, nc.gpsimd, nc.scalar]
                for mi in range(4):
                    cs = slice(mi * 128, (mi + 1) * 128)
                    gs = slice(jq * 512 + mi * 128, jq * 512 + (mi + 1) * 128)
                    nc.vector.tensor_mul(oT[t][0:64, gs], evicted[0][0:64, cs], bcs[0][:, cs])
                    tmp = znp.tile([64, 128], BT16, tag="tmp_os", bufs=4, name=f"tmsF{t}_{jq}_{mi}")
                    nc.vector.tensor_mul(tmp, evicted[1][0:64, cs], bcs[1][:, cs])
                    for n in range(2):
                        p3_final_unit(4 * jq + mi, n, tmp, dmae=rings[(2 * mi + n) % 3])

            # ---- main loop ----
            pend = []          # (t, jq, evicted) not yet normalized
            for jq in range(NJQ):
                flush_chunk(jq)
                for t in range(NP):
                    if pend:
                        pt_, pjq_, pev_ = pend.pop(0)
                        normalize(pt_, pjq_, pev_)
                        if pt_ == NP - 1 and pjq_ < NJQ - 1:
                            for m in range(4 * pjq_, 4 * pjq_ + 4):
                                for n in range(2):
                                    filler.append((None, lambda m=m, n=n, pjq_=pjq_: p3_unit(pjq_, m, n)))
                        if pt_ == NP - 2 and pjq_ == NJQ - 1:
                            units = [(m, n) for m in range(4 * pjq_, 4 * pjq_ + 4)
                                     for n in range(2)]
                            for m, n in units[:5]:
                                filler.append((None, lambda m=m, n=n: p3_part_unit(m, n)))
                            for m, n in units[5:]:
                                held.append(lambda m=m, n=n: p3_part_unit(m, n))
                    if jq == 0:
                        q_unit(0, t, xt0)
                        k_unit(0, t, xt0)
                        if t == 0:
                            v_unit(0, 0, xt0)
                            v_unit(0, 1, xt0)
                            for s in (2, 3):
                                filler.append((0, lambda s=s: v_unit(0, s, xt0)))
                    ev = attention(t, jq)
                    pend.append((t, jq, ev))
                    if jq == 0 and t == 0:
                        phase1_enqueue(1)
                if jq == 0:
                    phase1_enqueue(2)
                if jq == 1:
                    phase1_enqueue(3)
            drain(max(0, len(filler) - 3))
            pt_, pjq_, pev_ = pend.pop(0)
            normalize_final(pt_, pjq_, pev_)

    nc.compile()
    return nc


def _host_prep(x, wq, bq, wk, bk, wv, wo):
    def pack_w(w):
        # [(k p), hd] -> [p, k, hd]: contiguous 8KB per partition per DMA
        return np.ascontiguousarray(
            w.reshape(KC, 128, HD).transpose(1, 0, 2)).astype(BF16)

    masks_np = np.zeros((128, 4, 512), dtype=BF16)
    qn = np.arange(512)[None, :]
    kn = np.arange(128)[:, None]
    for d in range(4):
        masks_np[:, d, :] = (qn >= kn + 128 * d).astype(BF16)

    per_g = []
    for g in range(G):
        cs = slice(g * HD, (g + 1) * HD)
        per_g.append({
            "wq": pack_w(wq[:, cs]),
            "wk": pack_w(wk[:, cs]),
            "wv": pack_w(wv[:, cs]),
            "wo": np.ascontiguousarray(
                wo[cs, :].reshape(NP, 128, C).transpose(1, 0, 2)).astype(BF16),
            "wo3h": np.ascontiguousarray(wo[cs, :][7 * 64:8 * 64, :]).astype(BF16),
            "bq": np.ascontiguousarray((bq[cs] / 8.0).reshape(NP, 128).T).astype(np.float32),
            "bk": np.ascontiguousarray(bk[cs].reshape(NP, 128).T).astype(np.float32),
            "masks": masks_np,
        })
    in_maps = []
    for c in range(8):
        b, g = divmod(c, G)
        m = dict(per_g[g])
        xt = x[b].T.reshape(KC, 128, NJQ, 512).transpose(2, 1, 0, 3)
        m["xT"] = np.ascontiguousarray(xt).astype(BF16)
        in_maps.append(m)
    return in_maps


def kernel(x, wq, bq, wk, bk, wv, bv, wo, bo):
    x = np.asarray(x, dtype=np.float32)
    wq = np.asarray(wq, dtype=np.float32)
    bq = np.asarray(bq, dtype=np.float32)
    wk = np.asarray(wk, dtype=np.float32)
    bk = np.asarray(bk, dtype=np.float32)
    wv = np.asarray(wv, dtype=np.float32)
    bv = np.asarray(bv, dtype=np.float32)
    wo = np.asarray(wo, dtype=np.float32)
    bo = np.asarray(bo, dtype=np.float32)

    if "nc" not in _CACHED:
        _CACHED["nc"] = _build()
    nc = _CACHED["nc"]

    in_maps = _host_prep(x, wq, bq, wk, bk, wv, wo)
    res = run_bass_kernel_spmd(nc, in_maps, core_ids=list(range(8)))

    const_row = (bo.astype(np.float64) + bv.astype(np.float64) @ wo.astype(np.float64))
    out = np.empty((B, T, C), dtype=np.float32)
    for b in range(B):
        acc = res.results[2 * b]["y"].astype(np.float64)
        acc += res.results[2 * b + 1]["y"]
        acc += const_row[None, :]
        out[b] = acc.astype(np.float32)
    return out


# revision 24
# speedup vs baseline: 1.1623x; 1.1623x over previous
"""Causal self-attention (B=4, T=2048, C=1024, H=16, D=64) on 8 trn2 NeuronCores.

Sharding: core c = (batch b = c//2, head-group g = c%2). Megatron-style within a
batch: each core computes 8 heads' q/k/v (column-parallel) and a row-parallel
partial out-projection. Host sums the two partials per batch and adds the
rank-1 bias term (bo + bv @ wo) -- valid because softmax rows sum to 1, so v's
bias never needs to enter the kernel.

Per-core kernel (all matmuls bf16, fp32 PSUM accumulation):
  phase 1 (per 512-wide T chunk): qT,kT = (x@w)^T via lhsT=w, rhs=x^T (host
           pre-transposes x); v natural via lhsT=x^T-chunk, rhs=wv; a ones
           column is appended to each head's v block.
  phase 2: flash-style streaming attention in S^T orientation:
           S^T[k,q] = kT.T @ qT (head pairs packed in PE row groups 0/64,
           two concurrent 64x128 row tiles); P^T = exp(S^T) (ScalarE,
           1/sqrt(D) folded into q), causal mask-multiply on diagonal tiles
           only; O^T accumulated via lhsT=v_tile, rhs=P^T; the ones column
           of v makes PSUM row 64 the softmax denominator Z for free.
  phase 3: y = O @ wo via lhsT=O^T (already the natural layout), rhs=wo.

Scheduling: the PE sequencer is FIFO, so the span is governed by program
order, not just dependencies. The attention EXP stream (ScalarE, ~930ns/ik)
is the pace-setter; per-ik the PE has ~300ns of slack. All projection /
out-projection work is therefore chopped into small "filler units" (one
PSUM accumulation group each) that are drained one-at-a-time between
attention iks, keeping both engines dense. Chunk-0 q/k/v is fused into the
jq=0 pair loop (ramp), and the final chunk's out-projection accumulates
t=0..2 into SBUF mid-attention so only the t=3 term remains in the tail.
S PSUM uses 3 rotating slots so S^T(ik) never waits on exp(ik-2).
"""
import numpy as np
import ml_dtypes

import concourse.tile as tile
from concourse import bacc, mybir
from concourse.bass_utils import run_bass_kernel_spmd

BF16 = ml_dtypes.bfloat16
F32 = mybir.dt.float32
BT16 = mybir.dt.bfloat16
AF = mybir.ActivationFunctionType
ALU = mybir.AluOpType

B, T, C, H, D = 4, 2048, 1024, 16, 64
G = 2              # head groups (cores per batch)
HL = H // G        # heads per core = 8
HD = HL * D        # local head dims = 512
NP = 4             # head pairs per core
NJQ = T // 512     # q chunks of 512 = 4
NIK = T // 128     # k tiles of 128 = 16
KC = C // 128      # contraction chunks = 8

_CACHED = {}


def _build():
    nc = bacc.Bacc("TRN2", debug=False)
    xT = nc.dram_tensor("xT", [NJQ, 128, KC, 512], BT16, kind="ExternalInput").ap()
    wq = nc.dram_tensor("wq", [128, KC, HD], BT16, kind="ExternalInput").ap()
    wk = nc.dram_tensor("wk", [128, KC, HD], BT16, kind="ExternalInput").ap()
    wv = nc.dram_tensor("wv", [128, KC, HD], BT16, kind="ExternalInput").ap()
    wo = nc.dram_tensor("wo", [128, NP, C], BT16, kind="ExternalInput").ap()
    wo3h = nc.dram_tensor("wo3h", [64, C], BT16, kind="ExternalInput").ap()
    bq = nc.dram_tensor("bq", [128, NP], F32, kind="ExternalInput").ap()
    bk = nc.dram_tensor("bk", [128, NP], F32, kind="ExternalInput").ap()
    masks = nc.dram_tensor("masks", [128, 4, 512], BT16, kind="ExternalInput").ap()
    rcp_dram = nc.dram_tensor("rcp_dram", [NJQ, 8, 512], BT16).ap()
    y = nc.dram_tensor("y", [T, C], BT16, kind="ExternalOutput").ap()

    with tile.TileContext(nc) as tc:
        with (
            tc.tile_pool(name="consts", bufs=1) as consts,
            tc.tile_pool(name="xt", bufs=3) as xtp,
            tc.tile_pool(name="qk", bufs=1) as qkp,
            tc.tile_pool(name="vp", bufs=1) as vp,
            tc.tile_pool(name="otp", bufs=1) as otp,
            tc.tile_pool(name="pt", bufs=6) as ptp,
            tc.tile_pool(name="ptmp", bufs=3) as ptmpp,
            tc.tile_pool(name="zn", bufs=3) as znp,
            tc.tile_pool(name="yst", bufs=4) as ystp,
            tc.tile_pool(name="ps", bufs=3, space="PSUM") as ps,
        ):
            # ---- constants (biases are tiny and gate evictions: load them first) ----
            bq_dma = consts.tile([128, NP], F32, tag="bq_dma")
            bq_sb = consts.tile([128, NP], F32, tag="bq")
            nc.sync.dma_start(bq_dma, bq)
            nc.vector.tensor_copy(bq_sb, bq_dma)
            bk_dma = consts.tile([128, NP], F32, tag="bk_dma")
            bk_sb = consts.tile([128, NP], F32, tag="bk")
            nc.sync.dma_start(bk_dma, bk)
            nc.vector.tensor_copy(bk_sb, bk_dma)
            wq_sb = consts.tile([128, KC, HD], BT16, tag="wq")
            xt0 = xtp.tile([128, KC, 512], BT16, tag="xt", name="xt_pre0")
            wk_sb = consts.tile([128, KC, HD], BT16, tag="wk")
            for h in range(4):
                ks = slice(2 * h, 2 * h + 2)
                nc.scalar.dma_start(wq_sb[:, ks, :], wq[:, ks, :])
                nc.sync.dma_start(xt0[:, ks, :], xT[0][:, ks, :])
                nc.gpsimd.dma_start(wk_sb[:, ks, :], wk[:, ks, :])
            wv_sb = consts.tile([128, KC, HD], BT16, tag="wv")
            for h in range(2):
                ks = slice(4 * h, 4 * h + 4)
                nc.gpsimd.dma_start(wv_sb[:, ks, :], wv[:, ks, :])
            masks_sb = consts.tile([128, 4, 512], BT16, tag="masks")
            nc.gpsimd.dma_start(masks_sb, masks)
            wo_sb = consts.tile([128, NP, C], BT16, tag="wo")
            for h in range(2):
                nc.scalar.dma_start(wo_sb[:, 2 * h:2 * h + 2, :], wo[:, 2 * h:2 * h + 2, :])
            wo3h_sb = consts.tile([64, C], BT16, tag="wo3h")
            nc.scalar.dma_start(wo3h_sb, wo3h)
            # ---- persistent activations ----
            qT = [qkp.tile([128, T], BT16, tag=f"qT{t}", name=f"qT{t}") for t in range(NP)]
            kT = [qkp.tile([128, T], BT16, tag=f"kT{t}", name=f"kT{t}") for t in range(NP)]
            v_sb = [vp.tile([128, HL * 65], BT16, tag=f"v{i}", name=f"v{i}") for i in range(NIK)]
            oT = [otp.tile([128, T], BT16, tag=f"oT{t}", name=f"oT{t}") for t in range(NP)]
            yacc = {}  # (m, n) -> SBUF partial sum over t=0..2 for the last chunk

            # ---- phase 1 building blocks ----
            def q_unit(jt, t, xt):
                p = ps.tile([128, 512], F32, tag="st", name=f"pq{jt}_{t}")
                for k in range(KC):
                    nc.tensor.matmul(
                        p, wq_sb[:, k, t * 128:(t + 1) * 128], xt[:, k, :],
                        start=(k == 0), stop=(k == KC - 1),
                    )
                nc.vector.tensor_scalar(
                    qT[t][:, jt * 512:(jt + 1) * 512], p,
                    0.125, bq_sb[:, t:t + 1], ALU.mult, ALU.add,
                )

            def k_unit(jt, t, xt):
                p = ps.tile([128, 512], F32, tag="st", name=f"pk{jt}_{t}")
                for k in range(KC):
                    nc.tensor.matmul(
                        p, wk_sb[:, k, t * 128:(t + 1) * 128], xt[:, k, :],
                        start=(k == 0), stop=(k == KC - 1),
                    )
                nc.vector.tensor_scalar_add(
                    kT[t][:, jt * 512:(jt + 1) * 512], p, bk_sb[:, t:t + 1]
                )

            def v_unit(jt, s, xt):
                ik = jt * 4 + s
                p = ps.tile([128, 512], F32, tag="st", name=f"pv{ik}")
                for k in range(KC):
                    nc.tensor.matmul(
                        p, xt[:, k, s * 128:(s + 1) * 128], wv_sb[:, k, :],
                        start=(k == 0), stop=(k == KC - 1),
                    )
                vg = v_sb[ik].rearrange("p (h c) -> p h c", c=65)
                nc.vector.tensor_copy(
                    vg[:, :, 0:64], p.rearrange("p (h c) -> p h c", c=64)
                )
                nc.vector.memset(vg[:, :, 64:65], 1.0)

            def xt_load(jt):
                xt = xtp.tile([128, KC, 512], BT16, tag="xt", name=f"xt{jt}")
                nc.sync.dma_start(xt[:, 0:4, :], xT[jt][:, 0:4, :])
                nc.sync.dma_start(xt[:, 4:8, :], xT[jt][:, 4:8, :])
                return xt

            # ---- phase 3 building blocks ----
            def p3_unit(jq3, m, n):
                p = ps.tile([128, 512], F32, tag="st", name=f"py{m}_{n}")
                for t in range(NP):
                    nc.tensor.matmul(
                        p, oT[t][:, m * 128:(m + 1) * 128],
                        wo_sb[:, t, n * 512:(n + 1) * 512],
                        start=(t == 0), stop=(t == NP - 1),
                    )
                ys = ystp.tile([128, 512], BT16, tag="y", name=f"ys{m}_{n}")
                nc.vector.tensor_copy(ys, p)
                nc.gpsimd.dma_start(
                    y[m * 128:(m + 1) * 128, n * 512:(n + 1) * 512], ys
                )

            def p3_part_unit(m, n):
                # last chunk: accumulate t=0..2 into SBUF while pair 3's
                # attention still runs; t=3's term lands in the tail
                p = ps.tile([128, 512], F32, tag="st", name=f"pyp{m}_{n}")
                for t in range(NP - 1):
                    nc.tensor.matmul(
                        p, oT[t][:, m * 128:(m + 1) * 128],
                        wo_sb[:, t, n * 512:(n + 1) * 512],
                        start=(t == 0), stop=(t == NP - 2),
                    )
                ya = ystp.tile([128, 512], F32, tag=f"ya{m}_{n}", bufs=1, name=f"ya{m}_{n}")
                nc.vector.tensor_copy(ya, p)
                yacc[(m, n)] = ya

            def p3_final_unit(m, n, tmp, dmae=None):
                # t=3 term: head 6 from oT (written lane-aligned), head 7
                # straight from the normalize tmp tile via a low-partition
                # copy of wo's h7 rows -- no partition-shift DMA in the tail
                p = ps.tile([128, 512], F32, tag="st", name=f"pyf{m}_{n}")
                t = NP - 1
                nc.tensor.matmul(
                    p, oT[t][0:64, m * 128:(m + 1) * 128],
                    wo_sb[0:64, t, n * 512:(n + 1) * 512],
                    start=True, stop=False,
                )
                nc.tensor.matmul(
                    p, tmp, wo3h_sb[:, n * 512:(n + 1) * 512],
                    start=False, stop=True,
                )
                ys = ystp.tile([128, 512], BT16, tag="y", name=f"ysf{m}_{n}")
                nc.vector.tensor_add(ys, yacc[(m, n)], p)
                (dmae or nc.gpsimd).dma_start(
                    y[m * 128:(m + 1) * 128, n * 512:(n + 1) * 512], ys
                )

            # ---- filler queue: (chunk_gate, fn); gate=None for phase3 ----
            filler = []
            held = []   # units reserved for the tail's DMA-latency window

            def drain(n):
                for _ in range(n):
                    if filler:
                        filler.pop(0)[1]()

            def flush_chunk(jt):
                i = 0
                while i < len(filler):
                    gate, fn = filler[i]
                    if gate is not None and gate <= jt:
                        filler.pop(i)
                        fn()
                    else:
                        i += 1

            def phase1_enqueue(jt):
                xt = xt_load(jt)
                for t in range(NP):
                    filler.append((jt, lambda t=t, xt=xt: q_unit(jt, t, xt)))
                    filler.append((jt, lambda t=t, xt=xt: k_unit(jt, t, xt)))
                for s in range(4):
                    filler.append((jt, lambda s=s, xt=xt: v_unit(jt, s, xt)))

            # ---- attention ----
            def av(t, ik, nik, pts, o_ps):
                pt, c0 = pts[ik]
                ptg = pt.rearrange("p (h q) -> p h q", q=512)
                for hh in range(2):
                    h = 2 * t + hh
                    nc.tensor.matmul(
                        o_ps[hh][:, c0:512], v_sb[ik][:, h * 65:h * 65 + 65],
                        ptg[:, hh, c0:512],
                        start=(ik == 0), stop=(ik == nik - 1),
                    )

            def attention(t, jq):
                nik = 4 * jq + 4
                o_ps = [
                    ps.tile([65, 512], F32, tag="ot", bufs=2, name=f"ops{t}_{jq}_{_h}")
                    for _h in range(2)
                ]
                pts = {}
                for ik in range(nik):
                    d = ik - 4 * jq
                    c0 = 128 * d if d > 0 else 0   # first potentially-valid column
                    st = ps.tile([128, 1024], F32, tag="st", name=f"st{t}_{jq}_{ik}")
                    stg = st.rearrange("p (h q) -> p h q", q=512)
                    for hh in range(2):
                        r = slice(hh * 64, hh * 64 + 64)
                        nc.tensor.matmul(
                            stg[:, hh, c0:512],
                            kT[t][r, ik * 128:(ik + 1) * 128],
                            qT[t][r, jq * 512 + c0:(jq + 1) * 512],
                            start=True, stop=True,
                        )
                    pt = ptp.tile([128, 1024], BT16, tag="pt", name=f"pt{t}_{jq}_{ik}")
                    ptg = pt.rearrange("p (h q) -> p h q", q=512)
                    if d >= 0:
                        ptm = ptmpp.tile([128, 1024], BT16, tag="ptmp", name=f"ptm{t}_{jq}_{ik}")
                        ptmg = ptm.rearrange("p (h q) -> p h q", q=512)
                        nc.scalar.activation(ptmg[:, :, c0:512], stg[:, :, c0:512], AF.Exp)
                        for hh in range(2):
                            nc.vector.tensor_mul(
                                ptg[:, hh, c0:512],
                                ptmg[:, hh, c0:512],
                                masks_sb[:, d, c0:512],
                            )
                    else:
                        nc.scalar.activation(pt, st, AF.Exp)
                    pts[ik] = (pt, c0)
                    if ik > 0:
                        av(t, ik - 1, nik, pts, o_ps)
                    if ik % 2 == 1 and (jq < 2 or ik >= 4):
                        drain(1 if jq >= 2 else 2)
                av(t, nik - 1, nik, pts, o_ps)
                # evict Z row + unnormalized O^T, freeing the PSUM accumulators
                out_h = []
                for hh in range(2):
                    ouz = znp.tile([65, 512], F32, tag="ouz", bufs=6, name=f"oz{t}_{jq}_{hh}")
                    nc.vector.tensor_copy(ouz[64:65, :], o_ps[hh][64:65, :])
                    nc.vector.tensor_copy(ouz[0:64, :], o_ps[hh][0:64, :])
                    out_h.append(ouz)
                return out_h

            import concourse.bass as bass_mod

            def normalize(t, jq, evicted):
                # Pack both heads' Z rows [1,512] as [8,64] each -> one [16,64]
                # reciprocal (64 elems/lane), then broadcast 1/Z via a DRAM
                # round-trip (partition-step-0 DMA reads are legal from DRAM).
                qs2 = slice(jq * 512, (jq + 1) * 512)
                zb = znp.tile([16, 64], F32, tag="zb", bufs=2, name=f"zb{t}_{jq}")
                for hh in range(2):
                    ouz = evicted[hh]
                    nc.sync.dma_start(
                        zb[8 * hh:8 * hh + 8, :],
                        ouz[64:65, :].rearrange("o (p q) -> o p q", p=8),
                    )
                rcp = znp.tile([16, 64], F32, tag="rcpb", bufs=2, name=f"rcp{t}_{jq}")
                nc.vector.reciprocal(rcp, zb)
                rcp16 = znp.tile([16, 64], BT16, tag="rcp16b", bufs=2, name=f"rcp16{t}_{jq}")
                nc.vector.tensor_copy(rcp16, rcp)
                for hh in range(2):
                    nc.sync.dma_start(
                        rcp_dram[jq, 2 * t + hh, :].rearrange("(p q) -> p q", p=8),
                        rcp16[8 * hh:8 * hh + 8, :],
                    )
                for hh in range(2):
                    ouz = evicted[hh]
                    bc_sb = znp.tile([64, 512], BT16, tag="bc_sb", bufs=3, name=f"bs{t}_{jq}_{hh}")
                    src = rcp_dram[jq, 2 * t + hh, :]
                    bcast = bass_mod.AP(
                        tensor=src.tensor, offset=src.offset,
                        ap=[[0, 64]] + [list(a) for a in src.ap],
                    )
                    nc.sync.dma_start(bc_sb, bcast)
                    if hh == 0:
                        nc.vector.tensor_mul(oT[t][0:64, qs2], ouz[0:64, :], bc_sb)
                    else:
                        tmp = znp.tile([64, 512], BT16, tag="tmp_o", bufs=2, name=f"tm{t}_{jq}")
                        nc.vector.tensor_mul(tmp, ouz[0:64, :], bc_sb)
                        nc.gpsimd.dma_start(oT[t][64:128, qs2], tmp)

            def normalize_final(t, jq, evicted):
                # Last pair of the last chunk: same 1/Z machinery as
                # normalize() (Ln/Exp live in different ACT table sets, so
                # the DMA round-trip beats two table switches), but with the
                # oT muls done per-128-col slice so each slice's t=3
                # out-projection matmul + y writeback pipelines behind the
                # partition-shift DMA of the previous slice; DMAs alternate
                # across the three queues.
                zb = znp.tile([16, 64], F32, tag="zb", bufs=2, name=f"zbF{t}_{jq}")
                for hh in range(2):
                    nc.scalar.dma_start(
                        zb[8 * hh:8 * hh + 8, :],
                        evicted[hh][64:65, :].rearrange("o (p q) -> o p q", p=8),
                    )
                rcp = znp.tile([16, 64], F32, tag="rcpb", bufs=2, name=f"rcpF{t}_{jq}")
                nc.vector.reciprocal(rcp, zb)
                rcp16 = znp.tile([16, 64], BT16, tag="rcp16b", bufs=2, name=f"rcp16F{t}_{jq}")
                nc.vector.tensor_copy(rcp16, rcp)
                for hh in range(2):
                    nc.scalar.dma_start(
                        rcp_dram[jq, 2 * t + hh, :].rearrange("(p q) -> p q", p=8),
                        rcp16[8 * hh:8 * hh + 8, :],
                    )
                drain(len(filler))
                for fn in held:   # PE work covering the DMA latency window
                    fn()
                held.clear()
                bcs = []
                for hh in range(2):
                    bc_sb = znp.tile([64, 512], BT16, tag="bc_sb", bufs=3, name=f"bsF{t}_{jq}_{hh}")
                    src = rcp_dram[jq, 2 * t + hh, :]
                    bcast = bass_mod.AP(
                        tensor=src.tensor, offset=src.offset,
                        ap=[[0, 64]] + [list(a) for a in src.ap],
                    )
                    nc.scalar.dma_start(bc_sb, bcast)
                    bcs.append(bc_sb)
                rings = [nc.sync, nc.gpsimd, nc.scalar]
                for mi in range(4):
                    cs = slice(mi * 128, (mi + 1) * 128)
                    gs = slice(jq * 512 + mi * 128, jq * 512 + (mi + 1) * 128)
                    nc.vector.tensor_mul(oT[t][0:64, gs], evicted[0][0:64, cs], bcs[0][:, cs])
                    tmp = znp.tile([64, 128], BT16, tag="tmp_os", bufs=4, name=f"tmsF{t}_{jq}_{mi}")
                    nc.vector.tensor_mul(tmp, evicted[1][0:64, cs], bcs[1][:, cs])
                    for n in range(2):
                        p3_final_unit(4 * jq + mi, n, tmp, dmae=rings[(2 * mi + n) % 3])

            # ---- main loop ----
            pend = []          # (t, jq, evicted) not yet normalized
            for jq in range(NJQ):
                flush_chunk(jq)
                for t in range(NP):
                    if pend:
                        pt_, pjq_, pev_ = pend.pop(0)
                        normalize(pt_, pjq_, pev_)
                        if pt_ == NP - 1 and pjq_ < NJQ - 1:
                            for m in range(4 * pjq_, 4 * pjq_ + 4):
                                for n in range(2):
                                    filler.append((None, lambda m=m, n=n, pjq_=pjq_: p3_unit(pjq_, m, n)))
                        if pt_ == NP - 2 and pjq_ == NJQ - 1:
                            for m in range(4 * pjq_, 4 * pjq_ + 4):
                                for n in range(2):
                                    filler.append((None, lambda m=m, n=n: p3_part_unit(m, n)))
                    if jq == 0:
                        q_unit(0, t, xt0)
                        k_unit(0, t, xt0)
                        if t == 0:
                            v_unit(0, 0, xt0)
                            v_unit(0, 1, xt0)
                            for s in (2, 3):
                                filler.append((0, lambda s=s: v_unit(0, s, xt0)))
                    ev = attention(t, jq)
                    pend.append((t, jq, ev))
                    if jq == 0 and t == 0:
                        phase1_enqueue(1)
                if jq == 0:
                    phase1_enqueue(2)
                if jq == 1:
                    phase1_enqueue(3)
            drain(max(0, len(filler) - 3))
            pt_, pjq_, pev_ = pend.pop(0)
            normalize_final(pt_, pjq_, pev_)

    nc.compile()
    return nc


def _host_prep(x, wq, bq, wk, bk, wv, wo):
    def pack_w(w):
        # [(k p), hd] -> [p, k, hd]: contiguous 8KB per partition per DMA
        return np.ascontiguousarray(
            w.reshape(KC, 128, HD).transpose(1, 0, 2)).astype(BF16)

    masks_np = np.zeros((128, 4, 512), dtype=BF16)
    qn = np.arange(512)[None, :]
    kn = np.arange(128)[:, None]
    for d in range(4):
        masks_np[:, d, :] = (qn >= kn + 128 * d).astype(BF16)

    per_g = []
    for g in range(G):
        cs = slice(g * HD, (g + 1) * HD)
        per_g.append({
            "wq": pack_w(wq[:, cs]),
            "wk": pack_w(wk[:, cs]),
            "wv": pack_w(wv[:, cs]),
            "wo": np.ascontiguousarray(
                wo[cs, :].reshape(NP, 128, C).transpose(1, 0, 2)).astype(BF16),
            "wo3h": np.ascontiguousarray(wo[cs, :][7 * 64:8 * 64, :]).astype(BF16),
            "bq": np.ascontiguousarray((bq[cs] / 8.0).reshape(NP, 128).T).astype(np.float32),
            "bk": np.ascontiguousarray(bk[cs].reshape(NP, 128).T).astype(np.float32),
            "masks": masks_np,
        })
    in_maps = []
    for c in range(8):
        b, g = divmod(c, G)
        m = dict(per_g[g])
        xt = x[b].T.reshape(KC, 128, NJQ, 512).transpose(2, 1, 0, 3)
        m["xT"] = np.ascontiguousarray(xt).astype(BF16)
        in_maps.append(m)
    return in_maps


def kernel(x, wq, bq, wk, bk, wv, bv, wo, bo):
    x = np.asarray(x, dtype=np.float32)
    wq = np.asarray(wq, dtype=np.float32)
    bq = np.asarray(bq, dtype=np.float32)
    wk = np.asarray(wk, dtype=np.float32)
    bk = np.asarray(bk, dtype=np.float32)
    wv = np.asarray(wv, dtype=np.float32)
    bv = np.asarray(bv, dtype=np.float32)
    wo = np.asarray(wo, dtype=np.float32)
    bo = np.asarray(bo, dtype=np.float32)

    if "nc" not in _CACHED:
        _CACHED["nc"] = _build()
    nc = _CACHED["nc"]

    in_maps = _host_prep(x, wq, bq, wk, bk, wv, wo)
    res = run_bass_kernel_spmd(nc, in_maps, core_ids=list(range(8)))

    const_row = (bo.astype(np.float64) + bv.astype(np.float64) @ wo.astype(np.float64))
    out = np.empty((B, T, C), dtype=np.float32)
    for b in range(B):
        acc = res.results[2 * b]["y"].astype(np.float64)
        acc += res.results[2 * b + 1]["y"]
        acc += const_row[None, :]
        out[b] = acc.astype(np.float32)
    return out
